# revision 55
# baseline (speedup 1.0000x reference)
"""Trainium2 Bass kernel for LowRankRayTracer.

csi[f] = (delta_t/D) * v_f^T M v_f,  M = conj(rad)^T conj(att)  (R=32, complex)
contracted over N = D*K = 524288 rows.

Strategy (8 cores):
  - Shard the N rows across cores (512 directions each). csi is linear in M,
    so each core computes its partial S = rad^T att (128x128 f32; complex
    pairs via the f32 view + 2-rows-per-partition packing), builds
    W = [W_real|W_imag] in fp16, computes partial csi over ALL F=8192
    subcarriers, and the host sums the 8 partial csi vectors.
  - Precision budget: harness gate is rel_err < 2e-2, fp16-quantized inputs
    give ~5e-4, so rad/att/g/W/e are all fp16 "hi" only (no lo-correction
    passes): half the HBM bytes and a quarter of the PE columns vs the
    hi/lo-exact version.
  - Every load is split into multiple dma_starts to spread bytes evenly over
    the 16 DMA queues (one dma_start lands on one ~20 GB/s queue); io bufs=8
    keeps all of them in flight so the queues never starve.
  - PSUM-drain ops cost ~0.6us each regardless of size, so phase 3 uses
    two-bank-wide [128,1024] T tiles (8 DVE e-muls instead of 16); csi
    results are drained by the otherwise-idle ACT engine while the DVE
    does the e-muls (the PE sustains ~1.2 GHz under load and is the
    phase-3 pacer; DVE/ACT hide underneath).
"""

import numpy as np

D, K, R = 4096, 128, 32
F = 8192
N_CORES = 8
DIR_PER_CORE = D // N_CORES              # 512
N_MACRO = 8                              # macro tiles per tensor per core
MACRO_COLS = 4096                        # fp16 per partition per macro tile
SLICE = 128                              # matmul slice width (2 rows/partition)
SCALE = (200.0 / K) / D                  # delta_t / num_directions (exact binary)
FCHUNK = 512                             # one PSUM bank of f32 columns
N_UNIT = 4                               # phase-3 quad-bank units (2048 f)
NB = 2                                   # round-robin PSUM accumulator banks

_NC_CACHE = {}


def _build_consts():
    """(128, 258) f32: four (128,64) selection matrices + ones-selector cols."""
    c = np.zeros((128, 258), np.float32)
    EA = np.zeros((128, 32), np.float32)
    OA = np.zeros((128, 32), np.float32)
    EB = np.zeros((128, 32), np.float32)
    OB = np.zeros((128, 32), np.float32)
    for m in range(32):
        EA[2 * m, m] = 1.0
        OA[2 * m + 1, m] = 1.0
        EB[64 + 2 * m, m] = 1.0
        OB[64 + 2 * m + 1, m] = 1.0
    c[:, 0:32] = EA
    c[:, 32:64] = OA
    c[:, 64:96] = EB
    c[:, 96:128] = OB
    c[:, 128:160] = OA
    c[:, 160:192] = EA
    c[:, 192:224] = OB
    c[:, 224:256] = EB
    c[0:64, 256] = 1.0
    c[64:128, 257] = 1.0
    return c


def build_nc(n_macro=N_MACRO):
    import concourse.bacc as bacc
    import concourse.mybir as mybir
    import concourse.tile as tile

    fp32 = mybir.dt.float32
    fp16 = mybir.dt.float16
    nc = bacc.Bacc(trn_type="TRN2", target_bir_lowering=False, debug=False)

    rad_d = nc.dram_tensor("rad_h", [n_macro, 128, MACRO_COLS], fp16,
                           kind="ExternalInput").ap()
    att_d = nc.dram_tensor("att_h", [n_macro, 128, MACRO_COLS], fp16,
                           kind="ExternalInput").ap()
    gth_d = nc.dram_tensor("gth", [64, F], fp16, kind="ExternalInput").ap()
    cst_d = nc.dram_tensor("consts", [128, 258], fp32, kind="ExternalInput").ap()
    # output is the elementwise tensor e = g (*) (W^T g); the final
    # 128-partition fold into csi_re/csi_im rides along with the host's
    # existing 8-core partial summation
    out_d = nc.dram_tensor("e_out", [128, F], fp16, kind="ExternalOutput").ap()

    with tile.TileContext(nc) as tc:
        with (
            # bufs=8: all macros resident in SBUF so every bulk dma_start
            # issues immediately and the 16 queues stay fed
            tc.tile_pool(name="io", bufs=8) as io_pool,
            tc.tile_pool(name="small", bufs=1) as small,
            tc.tile_pool(name="epool", bufs=8) as epool,
        ):
            c_sb = small.tile([128, 258], fp32, tag="consts")
            nc.sync.dma_start(c_sb[:], cst_d[:])
            g2_sb = small.tile([128, F], fp16, tag="g2")

            # ---- main loop: S += rad^T att, fp16 hi-only ----
            s_sb = small.tile([128, 128], fp32, tag="s_sb")
            n_slices = MACRO_COLS // SLICE
            total = n_macro * n_slices
            with tc.tile_pool(name="spsum", bufs=1, space="PSUM") as spsum:
                banks = [spsum.tile([128, 512], fp32, tag=f"s{b}",
                                    name=f"sbank{b}")
                         for b in range(NB)]
                seen = [False] * NB
                idx = 0
                for i in range(n_macro):
                    rad = io_pool.tile([128, MACRO_COLS], fp16, tag="rad")
                    att = io_pool.tile([128, MACRO_COLS], fp16, tag="att")
                    # chunked loads: spread bytes over many DMA queues, and
                    # let the first matmuls start after ~0.5 MiB, not 2 MiB;
                    # fine chunks for the last macro too (less tail skew)
                    nch = 4 if i in (0, n_macro - 1) else 2
                    cm = MACRO_COLS // nch
                    for q in range(nch):
                        qs = slice(q * cm, (q + 1) * cm)
                        nc.sync.dma_start(rad[:, qs], rad_d[i, :, qs])
                        nc.scalar.dma_start(att[:, qs], att_d[i, :, qs])
                    if i == 1:
                        # g loads (1 MB) interleaved mid-loop: 4 chunked
                        # dma_starts into partitions 0:64
                        for q in range(4):
                            fs = slice(q * (F // 4), (q + 1) * (F // 4))
                            nc.sync.dma_start(g2_sb[0:64, fs], gth_d[:, fs])
                    if i == 2:
                        # duplicate g onto partitions 64:128 with the
                        # otherwise-idle DVE instead of re-reading HBM;
                        # all quarters issued here: a DVE op placed later in
                        # the macro loop also RUNS later (program position
                        # drives the schedule), which would push the last
                        # copy into the epilogue's critical path
                        for q in range(4):
                            fs = slice(q * (F // 4), (q + 1) * (F // 4))
                            nc.vector.tensor_copy(g2_sb[64:128, fs],
                                                  g2_sb[0:64, fs])
                    for s in range(n_slices):
                        sl = slice(s * SLICE, (s + 1) * SLICE)
                        b = idx % NB
                        nc.tensor.matmul(
                            banks[b][:, 0:SLICE],
                            lhsT=rad[:, sl],
                            rhs=att[:, sl],
                            start=not seen[b],
                            stop=(idx >= total - NB),
                        )
                        seen[b] = True
                        idx += 1

                # S = sum of the round-robin banks (Vector only: GpSimd
                # cannot access PSUM, and DVE reads max one PSUM operand)
                nc.vector.tensor_copy(s_sb[:], banks[0][:, 0:SLICE])
                for b in range(1, NB):
                    nc.vector.tensor_add(s_sb[:], s_sb[:], banks[b][:, 0:SLICE])

            # ---- epilogue: W = [W_real | W_imag] (64, 128) fp16, built as
            # add/subs of the pre-scaled selection matmul outputs ----
            with tc.tile_pool(name="vpsum", bufs=1, space="PSUM") as vpsum:
                v1 = vpsum.tile([64, 64], fp32, tag="v1")
                nc.tensor.matmul(v1[:], lhsT=c_sb[:, 0:64], rhs=s_sb[:, 0:64],
                                 start=True, stop=False)
                nc.tensor.matmul(v1[:], lhsT=c_sb[:, 64:128],
                                 rhs=s_sb[:, 64:128], start=False, stop=True)
                v2 = vpsum.tile([64, 64], fp32, tag="v2")
                nc.tensor.matmul(v2[:], lhsT=c_sb[:, 128:192],
                                 rhs=s_sb[:, 0:64], start=True, stop=False)
                nc.tensor.matmul(v2[:], lhsT=c_sb[:, 192:256],
                                 rhs=s_sb[:, 64:128], start=False, stop=True)

                s_ = float(SCALE)
                a_sb = small.tile([64, 64], fp32, tag="a_sb")   # v1 * s
                b_sb = small.tile([64, 64], fp32, tag="b_sb")   # v2 * -s
                c2_sb = small.tile([64, 64], fp32, tag="c2_sb")  # v2 * s
                nc.vector.tensor_scalar_mul(a_sb[:], v1[:], s_)
                nc.vector.tensor_scalar_mul(b_sb[:], v2[:], -s_)
                # ACT can read PSUM and is otherwise idle here
                nc.scalar.mul(c2_sb[:], v2[:], s_)

            # W quadrants in fp16 directly. With a = v1*s, b = -v2*s,
            # c2 = v2*s and the dup-stacked row ranges:
            #   rows 0:32 : Mr*s = a_e + b_o, -Mi*s = a_o - b_e, Mi*s = b_e - a_o
            #   rows 32:64: -Mi*s = a_e - b_o, -Mr*s = b_e + a_o, Mr*s = c2_e - a_o
            # GpSimd takes the three same-type SUBs (one library reload).
            wh = small.tile([64, 128], fp16, tag="wh")
            r1, r2 = slice(0, 32), slice(32, 64)
            ev, od = slice(0, 64, 2), slice(1, 64, 2)
            # W_real = [[Mr, -Mi], [-Mi, -Mr]] * s
            nc.vector.tensor_add(wh[r1, 0:32], a_sb[r1, ev], b_sb[r1, od])
            nc.vector.tensor_sub(wh[r1, 32:64], a_sb[r1, od], b_sb[r1, ev])
            nc.gpsimd.tensor_sub(wh[r2, 0:32], a_sb[r2, ev], b_sb[r2, od])
            nc.vector.tensor_add(wh[r2, 32:64], b_sb[r2, ev], a_sb[r2, od])
            # W_imag = [[Mi, Mr], [Mr, -Mi]] * s
            nc.vector.tensor_sub(wh[r1, 64:96], b_sb[r1, ev], a_sb[r1, od])
            nc.vector.tensor_add(wh[r1, 96:128], a_sb[r1, ev], b_sb[r1, od])
            nc.gpsimd.tensor_sub(wh[r2, 64:96], c2_sb[r2, ev], a_sb[r2, od])
            nc.gpsimd.tensor_sub(wh[r2, 96:128], a_sb[r2, ev], b_sb[r2, od])

            # ---- phase 3: e = g (*) (W^T g) over F in 4 quad-bank units
            # (all 8 PSUM banks; DVE PSUM-drain ops cost ~330ns each in
            # overhead, so fewer, bigger e-muls pace faster). Each unit's e
            # streams straight out; one dma_start is one ~20 GB/s queue and
            # one sequencer issue is ~0.4us, so pieces balance both. ----
            with tc.tile_pool(name="tpsum", bufs=2, space="PSUM") as tpsum:
                t_tiles = [tpsum.tile([128, 4 * FCHUNK], fp32, tag="t",
                                      name=f"t{k}") for k in range(2)]

                for u in range(N_UNIT):
                    t_ps = t_tiles[u % 2]
                    for h in range(4):
                        fs = slice((4 * u + h) * FCHUNK,
                                   (4 * u + h + 1) * FCHUNK)
                        # T = W^T g, fp16 single pass; out stays in one bank
                        nc.tensor.matmul(t_ps[:, h * FCHUNK:(h + 1) * FCHUNK],
                                         lhsT=wh[:], rhs=g2_sb[0:64, fs],
                                         start=True, stop=True)
                    e_sb = epool.tile([128, 4 * FCHUNK], fp16, tag="e",
                                      name=f"e{u}")
                    us = 4 * u * FCHUNK
                    # one DVE drain per four banks
                    nc.vector.tensor_mul(e_sb[:], g2_sb[:, us:us + 4 * FCHUNK],
                                         t_ps[:])
                    # 64 KB out pieces; 32 KB on the last unit for the tail
                    np_ = 8 if u < N_UNIT - 1 else 16
                    qc = 4 * FCHUNK // np_
                    for q in range(np_):
                        eng = (nc.sync, nc.scalar, nc.gpsimd)[q % 3]
                        eng.dma_start(
                            out_d[:, us + q * qc:us + (q + 1) * qc],
                            e_sb[:, q * qc:(q + 1) * qc])

    nc.compile()
    return nc


def _prep_shared(fbv):
    """gth (64, F) fp16 from complex fbv (F, R): rows = [Re ranks; Im ranks]."""
    fbv32 = np.ascontiguousarray(fbv).view(np.float32).reshape(F, 2 * R)
    gbt = np.ascontiguousarray(
        np.concatenate([fbv32[:, 0::2].T, fbv32[:, 1::2].T], axis=0))
    return gbt.astype(np.float16)


def _shard_h(arr, core):
    """Core's complex64 shard -> fp16 hi array (N_MACRO, 128, MACRO_COLS)."""
    sh = arr[core * DIR_PER_CORE:(core + 1) * DIR_PER_CORE]
    f32 = np.ascontiguousarray(sh).view(np.float32).ravel()
    return f32.astype(np.float16).reshape(N_MACRO, 128, MACRO_COLS)


def kernel(attenuation_vectors, radiation_vectors, frequency_basis_vectors):
    from concourse.bass_utils import run_bass_kernel_spmd

    if "nc" not in _NC_CACHE:
        _NC_CACHE["nc"] = build_nc()
    nc = _NC_CACHE["nc"]

    gth = _prep_shared(frequency_basis_vectors)
    consts = _build_consts()
    in_maps = []
    for c in range(N_CORES):
        in_maps.append({
            "rad_h": _shard_h(radiation_vectors, c),
            "att_h": _shard_h(attenuation_vectors, c),
            "gth": gth,
            "consts": consts,
        })

    res = run_bass_kernel_spmd(nc, in_maps, core_ids=list(range(N_CORES)))
    # fold the per-core e tensors: csi_re[f] = sum_i<64 e[i,f],
    # csi_im[f] = sum_i>=64 e[i,f], summed over the 8 cores
    acc = np.zeros((2, F), np.float64)
    for r in res.results:
        e = r["e_out"].astype(np.float64)
        acc[0] += e[0:64].sum(axis=0)
        acc[1] += e[64:128].sum(axis=0)
    return (acc[0] + 1j * acc[1]).astype(np.complex64)


# revision 57
# speedup vs baseline: 1.0892x; 1.0892x over previous
"""Trainium2 Bass kernel for LowRankRayTracer.

csi[f] = (delta_t/D) * v_f^T M v_f,  M = conj(rad)^T conj(att)  (R=32, complex)
contracted over N = D*K = 524288 rows.

Strategy (8 cores):
  - Shard the N rows across cores (512 directions each). csi is linear in M,
    so each core computes its partial S = rad^T att (128x128 f32; complex
    pairs via the f32 view + 2-rows-per-partition packing), builds
    W = [W_real|W_imag] in fp16, computes partial csi over ALL F=8192
    subcarriers, and the host sums the 8 partial csi vectors.
  - Precision budget: harness gate is rel_err < 2e-2, fp16-quantized inputs
    give ~5e-4, so rad/att/g/W/e are all fp16 "hi" only (no lo-correction
    passes): half the HBM bytes and a quarter of the PE columns vs the
    hi/lo-exact version.
  - Every load is split into multiple dma_starts to spread bytes evenly over
    the 16 DMA queues (one dma_start lands on one ~20 GB/s queue); io bufs=8
    keeps all of them in flight so the queues never starve.
  - PSUM-drain ops cost ~0.6us each regardless of size, so phase 3 uses
    two-bank-wide [128,1024] T tiles (8 DVE e-muls instead of 16); csi
    results are drained by the otherwise-idle ACT engine while the DVE
    does the e-muls (the PE sustains ~1.2 GHz under load and is the
    phase-3 pacer; DVE/ACT hide underneath).
"""

import numpy as np

D, K, R = 4096, 128, 32
F = 8192
N_CORES = 8
DIR_PER_CORE = D // N_CORES              # 512
N_MACRO = 8                              # macro tiles per tensor per core
MACRO_COLS = 4096                        # fp16 per partition per macro tile
SLICE = 128                              # matmul slice width (2 rows/partition)
SCALE = (200.0 / K) / D                  # delta_t / num_directions (exact binary)
FCHUNK = 512                             # one PSUM bank of f32 columns
N_UNIT = 8                               # phase-3 double-bank units (1024 f)
NB = 2                                   # round-robin PSUM accumulator banks

_NC_CACHE = {}


def _build_consts():
    """(128, 258) f32: four (128,64) selection matrices + ones-selector cols."""
    c = np.zeros((128, 258), np.float32)
    EA = np.zeros((128, 32), np.float32)
    OA = np.zeros((128, 32), np.float32)
    EB = np.zeros((128, 32), np.float32)
    OB = np.zeros((128, 32), np.float32)
    for m in range(32):
        EA[2 * m, m] = 1.0
        OA[2 * m + 1, m] = 1.0
        EB[64 + 2 * m, m] = 1.0
        OB[64 + 2 * m + 1, m] = 1.0
    c[:, 0:32] = EA
    c[:, 32:64] = OA
    c[:, 64:96] = EB
    c[:, 96:128] = OB
    c[:, 128:160] = OA
    c[:, 160:192] = EA
    c[:, 192:224] = OB
    c[:, 224:256] = EB
    c[0:64, 256] = 1.0
    c[64:128, 257] = 1.0
    return c


def build_nc(n_macro=N_MACRO):
    import concourse.bacc as bacc
    import concourse.mybir as mybir
    import concourse.tile as tile

    fp32 = mybir.dt.float32
    fp16 = mybir.dt.float16
    nc = bacc.Bacc(trn_type="TRN2", target_bir_lowering=False, debug=False)

    rad_d = nc.dram_tensor("rad_h", [n_macro, 128, MACRO_COLS], fp16,
                           kind="ExternalInput").ap()
    att_d = nc.dram_tensor("att_h", [n_macro, 128, MACRO_COLS], fp16,
                           kind="ExternalInput").ap()
    gth_d = nc.dram_tensor("gth", [64, F], fp16, kind="ExternalInput").ap()
    cst_d = nc.dram_tensor("consts", [128, 258], fp32, kind="ExternalInput").ap()
    # output is the elementwise tensor e = g (*) (W^T g); the final
    # 128-partition fold into csi_re/csi_im rides along with the host's
    # existing 8-core partial summation
    out_d = nc.dram_tensor("e_out", [128, F], fp16, kind="ExternalOutput").ap()

    with tile.TileContext(nc) as tc:
        with (
            # bufs=8: all macros resident in SBUF so every bulk dma_start
            # issues immediately and the 16 queues stay fed
            tc.tile_pool(name="io", bufs=8) as io_pool,
            tc.tile_pool(name="small", bufs=1) as small,
            tc.tile_pool(name="epool", bufs=8) as epool,
        ):
            c_sb = small.tile([128, 258], fp32, tag="consts")
            nc.sync.dma_start(c_sb[:], cst_d[:])
            g2_sb = small.tile([128, F], fp16, tag="g2")

            # ---- main loop: S += rad^T att, fp16 hi-only ----
            s_sb = small.tile([128, 128], fp32, tag="s_sb")
            n_slices = MACRO_COLS // SLICE
            total = n_macro * n_slices
            with tc.tile_pool(name="spsum", bufs=1, space="PSUM") as spsum:
                banks = [spsum.tile([128, 512], fp32, tag=f"s{b}",
                                    name=f"sbank{b}")
                         for b in range(NB)]
                seen = [False] * NB
                idx = 0
                for i in range(n_macro):
                    rad = io_pool.tile([128, MACRO_COLS], fp16, tag="rad")
                    att = io_pool.tile([128, MACRO_COLS], fp16, tag="att")
                    # chunked loads: spread bytes over many DMA queues, and
                    # let the first matmuls start after ~0.5 MiB, not 2 MiB;
                    # fine chunks for the last macro too (less tail skew)
                    nch = 4 if i in (0, n_macro - 1) else 2
                    cm = MACRO_COLS // nch
                    for q in range(nch):
                        qs = slice(q * cm, (q + 1) * cm)
                        nc.sync.dma_start(rad[:, qs], rad_d[i, :, qs])
                        nc.scalar.dma_start(att[:, qs], att_d[i, :, qs])
                    if i == 1:
                        # g loads (1 MB) interleaved mid-loop: 4 chunked
                        # dma_starts into partitions 0:64
                        for q in range(4):
                            fs = slice(q * (F // 4), (q + 1) * (F // 4))
                            nc.sync.dma_start(g2_sb[0:64, fs], gth_d[:, fs])
                    if i == 2:
                        # duplicate g onto partitions 64:128 with the
                        # otherwise-idle DVE instead of re-reading HBM;
                        # all quarters issued here: a DVE op placed later in
                        # the macro loop also RUNS later (program position
                        # drives the schedule), which would push the last
                        # copy into the epilogue's critical path
                        for q in range(4):
                            fs = slice(q * (F // 4), (q + 1) * (F // 4))
                            nc.vector.tensor_copy(g2_sb[64:128, fs],
                                                  g2_sb[0:64, fs])
                    for s in range(n_slices):
                        sl = slice(s * SLICE, (s + 1) * SLICE)
                        b = idx % NB
                        nc.tensor.matmul(
                            banks[b][:, 0:SLICE],
                            lhsT=rad[:, sl],
                            rhs=att[:, sl],
                            start=not seen[b],
                            stop=(idx >= total - NB),
                        )
                        seen[b] = True
                        idx += 1

                # S = sum of the round-robin banks (Vector only: GpSimd
                # cannot access PSUM, and DVE reads max one PSUM operand)
                nc.vector.tensor_copy(s_sb[:], banks[0][:, 0:SLICE])
                for b in range(1, NB):
                    nc.vector.tensor_add(s_sb[:], s_sb[:], banks[b][:, 0:SLICE])

            # ---- epilogue: W = [W_real | W_imag] (64, 128) fp16, built as
            # add/subs of the pre-scaled selection matmul outputs ----
            with tc.tile_pool(name="vpsum", bufs=1, space="PSUM") as vpsum:
                v1 = vpsum.tile([64, 64], fp32, tag="v1")
                nc.tensor.matmul(v1[:], lhsT=c_sb[:, 0:64], rhs=s_sb[:, 0:64],
                                 start=True, stop=False)
                nc.tensor.matmul(v1[:], lhsT=c_sb[:, 64:128],
                                 rhs=s_sb[:, 64:128], start=False, stop=True)
                v2 = vpsum.tile([64, 64], fp32, tag="v2")
                nc.tensor.matmul(v2[:], lhsT=c_sb[:, 128:192],
                                 rhs=s_sb[:, 0:64], start=True, stop=False)
                nc.tensor.matmul(v2[:], lhsT=c_sb[:, 192:256],
                                 rhs=s_sb[:, 64:128], start=False, stop=True)

                s_ = float(SCALE)
                a_sb = small.tile([64, 64], fp32, tag="a_sb")   # v1 * s
                b_sb = small.tile([64, 64], fp32, tag="b_sb")   # v2 * -s
                c2_sb = small.tile([64, 64], fp32, tag="c2_sb")  # v2 * s
                nc.vector.tensor_scalar_mul(a_sb[:], v1[:], s_)
                nc.vector.tensor_scalar_mul(b_sb[:], v2[:], -s_)
                # ACT can read PSUM and is otherwise idle here
                nc.scalar.mul(c2_sb[:], v2[:], s_)

            # W quadrants in fp16 directly. With a = v1*s, b = -v2*s,
            # c2 = v2*s and the dup-stacked row ranges:
            #   rows 0:32 : Mr*s = a_e + b_o, -Mi*s = a_o - b_e, Mi*s = b_e - a_o
            #   rows 32:64: -Mi*s = a_e - b_o, -Mr*s = b_e + a_o, Mr*s = c2_e - a_o
            # GpSimd takes the three same-type SUBs (one library reload).
            wh = small.tile([64, 128], fp16, tag="wh")
            r1, r2 = slice(0, 32), slice(32, 64)
            ev, od = slice(0, 64, 2), slice(1, 64, 2)
            # W_real = [[Mr, -Mi], [-Mi, -Mr]] * s
            nc.vector.tensor_add(wh[r1, 0:32], a_sb[r1, ev], b_sb[r1, od])
            nc.vector.tensor_sub(wh[r1, 32:64], a_sb[r1, od], b_sb[r1, ev])
            nc.gpsimd.tensor_sub(wh[r2, 0:32], a_sb[r2, ev], b_sb[r2, od])
            nc.vector.tensor_add(wh[r2, 32:64], b_sb[r2, ev], a_sb[r2, od])
            # W_imag = [[Mi, Mr], [Mr, -Mi]] * s
            nc.vector.tensor_sub(wh[r1, 64:96], b_sb[r1, ev], a_sb[r1, od])
            nc.vector.tensor_add(wh[r1, 96:128], a_sb[r1, ev], b_sb[r1, od])
            nc.gpsimd.tensor_sub(wh[r2, 64:96], c2_sb[r2, ev], a_sb[r2, od])
            nc.gpsimd.tensor_sub(wh[r2, 96:128], a_sb[r2, ev], b_sb[r2, od])

            # ---- phase 3: e = g (*) (W^T g) over F in 8 double-bank units.
            # The DVE e-muls (~1.2us/unit, PSUM-read-bound) are the pacer;
            # each unit's e streams straight out over several DMA queues
            # (one dma_start is one ~20 GB/s queue and one sequencer issue
            # is ~0.4us, so the piece count balances both limits). ----
            with tc.tile_pool(name="tpsum", bufs=2, space="PSUM") as tpsum:
                t_tiles = [tpsum.tile([128, 2 * FCHUNK], fp32, tag="t",
                                      name=f"t{k}") for k in range(2)]

                for u in range(N_UNIT):
                    t_ps = t_tiles[u % 2]
                    for h in range(2):
                        fs = slice((2 * u + h) * FCHUNK,
                                   (2 * u + h + 1) * FCHUNK)
                        # T = W^T g, fp16 single pass; out stays in one bank
                        nc.tensor.matmul(t_ps[:, h * FCHUNK:(h + 1) * FCHUNK],
                                         lhsT=wh[:], rhs=g2_sb[0:64, fs],
                                         start=True, stop=True)
                    e_sb = epool.tile([128, 2 * FCHUNK], fp16, tag="e",
                                      name=f"e{u}")
                    us = 2 * u * FCHUNK
                    # one DVE drain per two banks
                    nc.vector.tensor_mul(e_sb[:], g2_sb[:, us:us + 2 * FCHUNK],
                                         t_ps[:])
                    # stream e out in 64 KB pieces over 3 issuing engines;
                    # 32 KB pieces for the last two units to cut the tail
                    np_ = 4 if u < N_UNIT - 2 else 8
                    qc = 2 * FCHUNK // np_
                    for q in range(np_):
                        eng = (nc.sync, nc.scalar, nc.gpsimd)[q % 3]
                        eng.dma_start(
                            out_d[:, us + q * qc:us + (q + 1) * qc],
                            e_sb[:, q * qc:(q + 1) * qc])

    nc.compile()
    return nc


def _prep_shared(fbv):
    """gth (64, F) fp16 from complex fbv (F, R): rows = [Re ranks; Im ranks]."""
    fbv32 = np.ascontiguousarray(fbv).view(np.float32).reshape(F, 2 * R)
    gbt = np.ascontiguousarray(
        np.concatenate([fbv32[:, 0::2].T, fbv32[:, 1::2].T], axis=0))
    return gbt.astype(np.float16)


def _shard_h(arr, core):
    """Core's complex64 shard -> fp16 hi array (N_MACRO, 128, MACRO_COLS)."""
    sh = arr[core * DIR_PER_CORE:(core + 1) * DIR_PER_CORE]
    f32 = np.ascontiguousarray(sh).view(np.float32).ravel()
    return f32.astype(np.float16).reshape(N_MACRO, 128, MACRO_COLS)


def kernel(attenuation_vectors, radiation_vectors, frequency_basis_vectors):
    from concourse.bass_utils import run_bass_kernel_spmd

    if "nc" not in _NC_CACHE:
        _NC_CACHE["nc"] = build_nc()
    nc = _NC_CACHE["nc"]

    gth = _prep_shared(frequency_basis_vectors)
    consts = _build_consts()
    in_maps = []
    for c in range(N_CORES):
        in_maps.append({
            "rad_h": _shard_h(radiation_vectors, c),
            "att_h": _shard_h(attenuation_vectors, c),
            "gth": gth,
            "consts": consts,
        })

    res = run_bass_kernel_spmd(nc, in_maps, core_ids=list(range(N_CORES)))
    # fold the per-core e tensors: csi_re[f] = sum_i<64 e[i,f],
    # csi_im[f] = sum_i>=64 e[i,f], summed over the 8 cores
    acc = np.zeros((2, F), np.float64)
    for r in res.results:
        e = r["e_out"].astype(np.float64)
        acc[0] += e[0:64].sum(axis=0)
        acc[1] += e[64:128].sum(axis=0)
    return (acc[0] + 1j * acc[1]).astype(np.complex64)
